# revision 9
# baseline (speedup 1.0000x reference)
"""LoRA linear layer (out = x @ (W + B@A).T + bias) on 8 trn2 NeuronCores.

Strategy: data-parallel over tokens (B*S = 8192 -> 1024 tokens/core), with
all layout work hoisted to the host so the tensor engine runs a pure
LDWEIGHTS->matmul stream at the fp16 rate:
  - Host folds the LoRA delta into the weight (W' = W + B@A), transposes
    and block-packs W' so each 128-row output block DMAs straight into the
    stationary-operand layout [128 k-par, KT, 128 o], fp16.
  - Host transposes each core's token shard into x^T [128 k-par, KT, T],
    fp16 (half the DMA bytes of fp32), DMA'd once and SBUF-resident.
  - Device: for each of the 32 output blocks, accumulate psum[o=128, t=512]
    over the 32 k-tiles (fp16 matmuls, one stationary load per k reused for
    both t-chunks), evict through the Scalar engine with the bias added,
    and DMA the [o, t] fp16 tile out contiguously.
  - Host transposes the [d_out, T] per-core results back to [T, d_out].
"""

import sys

sys.path.insert(0, "/opt/trn_rl_repo")

import numpy as np

import concourse.bass as bass  # noqa: F401
import concourse.bacc as bacc
import concourse.tile as tile
from concourse import mybir, bass_utils
from contextlib import ExitStack

P = 128
N_CORES = 8

# Full problem shapes (hardcoded per contract).
B_FULL, S_FULL, D_IN, D_OUT, R = 4, 2048, 4096, 4096, 16
T = (B_FULL * S_FULL) // N_CORES  # 1024 tokens per core
KT = D_IN // P  # 32 contraction tiles
MT = D_OUT // P  # 32 output row-blocks
NCH = 512  # moving-operand chunk (psum bank width in fp32)
NT = T // NCH  # 2 chunks


def build_nc(**_):
    """Per-core bass program; all cores run it on different token shards."""
    FP = mybir.dt.float32
    F16 = mybir.dt.float16

    nc = bacc.Bacc("TRN2", target_bir_lowering=False, debug=False)
    xt_d = nc.dram_tensor("xt", [P, KT, T], F16, kind="ExternalInput").ap()
    wq_d = nc.dram_tensor("wq", [MT, P, KT, P], F16, kind="ExternalInput").ap()
    b_d = nc.dram_tensor("bias_r", [P, MT], FP, kind="ExternalInput").ap()
    out_d = nc.dram_tensor("out", [D_OUT, T], F16, kind="ExternalOutput").ap()

    with tile.TileContext(nc) as tc, ExitStack() as ctx:
        const = ctx.enter_context(tc.tile_pool(name="const", bufs=1))
        wt_pool = ctx.enter_context(tc.tile_pool(name="wtp", bufs=3))
        ps_pool = ctx.enter_context(tc.tile_pool(name="psp", bufs=4, space="PSUM"))
        ob_pool = ctx.enter_context(tc.tile_pool(name="obp", bufs=4))

        wts = {}

        def fetch_w(m):
            wt = wt_pool.tile([P, KT, P], F16, tag="wt", name=f"wt{m}")
            nc.scalar.dma_start(wt[:], wq_d[m])
            wts[m] = wt

        # First W block issued before anything else on the scalar queue so
        # the first LDWEIGHTS can go as early as possible.
        fetch_w(0)
        bias_sb = const.tile([P, MT], FP)
        nc.gpsimd.dma_start(bias_sb[:], b_d[:])
        # Resident x^T shard in 16 separate tiles so the first matmuls only
        # wait on a 512 KB chunk, not the whole 8 MB load.
        XCH = 16
        KC = KT // XCH
        xts = []
        for q in range(XCH):
            xq = const.tile([P, KC, T], F16, tag=f"xq{q}")
            nc.sync.dma_start(xq[:], xt_d[:, q * KC : (q + 1) * KC, :])
            xts.append(xq)

        def xt_slice(k, nsl):
            return xts[k // KC][:, k % KC, nsl]
        for m in range(MT):
            if m + 1 < MT:
                fetch_w(m + 1)
            wt = wts.pop(m)
            mps = [
                ps_pool.tile([P, NCH], FP, tag=f"mm{n}", name=f"mps{n}")
                for n in range(NT)
            ]
            for k in range(KT):
                for n in range(NT):
                    nc.tensor.matmul(
                        mps[n][:],
                        wt[:, k, :],
                        xt_slice(k, slice(n * NCH, (n + 1) * NCH)),
                        start=(k == 0),
                        stop=(k == KT - 1),
                    )
            for n in range(NT):
                ob = ob_pool.tile([P, NCH], F16, tag="ob", name="ob")
                nc.scalar.activation(
                    ob[:],
                    mps[n][:],
                    mybir.ActivationFunctionType.Identity,
                    bias=bias_sb[:, m : m + 1],
                )
                nc.sync.dma_start(
                    out_d[m * P : (m + 1) * P, n * NCH : (n + 1) * NCH], ob[:]
                )

    nc.compile()
    return nc


def make_in_maps(x, weight, bias, lora_A, lora_B):
    Wp = weight.astype(np.float32) + lora_B.astype(np.float32) @ lora_A.astype(
        np.float32
    )
    # wq[m, p, k, o] = Wp[m*128 + o, k*128 + p]  (stationary layout, fp16)
    wq = np.ascontiguousarray(
        Wp.reshape(MT, P, KT, P).transpose(0, 3, 2, 1).astype(np.float16)
    )
    bias_r = np.ascontiguousarray(bias.astype(np.float32).reshape(MT, P).T)
    xf = x.reshape(-1, D_IN).astype(np.float16)
    maps = []
    for c in range(N_CORES):
        xc = xf[c * T : (c + 1) * T]
        # xt[p, k, t] = x[t, k*128 + p]
        xt = np.ascontiguousarray(xc.reshape(T, KT, P).transpose(2, 1, 0))
        maps.append({"xt": xt, "wq": wq, "bias_r": bias_r})
    return maps


def assemble_out(results):
    """Per-core [d_out, T] fp16 -> full [B, S, d_out] fp32."""
    out = np.empty((B_FULL * S_FULL, D_OUT), dtype=np.float32)
    for c in range(N_CORES):
        out[c * T : (c + 1) * T] = results[c]["out"].T
    return out.reshape(B_FULL, S_FULL, D_OUT)


_nc_cache = {}


def kernel(x, weight, bias, lora_A, lora_B):
    key = (x.shape, weight.shape)
    if key not in _nc_cache:
        _nc_cache[key] = build_nc()
    nc = _nc_cache[key]
    in_maps = make_in_maps(x, weight, bias, lora_A, lora_B)
    res = bass_utils.run_bass_kernel_spmd(nc, in_maps, core_ids=list(range(N_CORES)))
    return assemble_out(res.results)


if __name__ == "__main__":
    rng = np.random.default_rng(0)
    x = rng.standard_normal((B_FULL, S_FULL, D_IN), dtype=np.float32)
    w = (rng.standard_normal((D_OUT, D_IN), dtype=np.float32) * 0.02).astype(np.float32)
    b = (rng.standard_normal((D_OUT,), dtype=np.float32) * 0.02).astype(np.float32)
    la = (rng.standard_normal((R, D_IN), dtype=np.float32) * 0.02).astype(np.float32)
    lb = (rng.standard_normal((D_OUT, R), dtype=np.float32) * 0.02).astype(np.float32)
    out = kernel(x, w, b, la, lb)
    ref = x.reshape(-1, D_IN) @ (w + lb @ la).T + b
    err = np.abs(out.reshape(-1, D_OUT) - ref)
    denom = np.abs(ref).max()
    print("max abs err:", err.max(), "rel:", err.max() / denom)
